# revision 7
# baseline (speedup 1.0000x reference)
"""GNN encoder (message passing) Bass kernel for 8 trn2 NeuronCores.

Strategy (edge-parallel, dst-sorted sharding):
  - Nodes padded to 50176 = 392 blocks of 128; 49 blocks per core.
  - Edges bucketed by dst block -> owning core computes the edge MLP and the
    local segment-sum for its own nodes; no cross-core reduction needed.
  - Per block, edge slots split into lo/hi phases by src < 32768 (dma_gather
    indices are int16), padded to uniform K_lo / K_hi columns of 128 slots so
    a single SPMD program serves all 8 cores.
  - hn0 node table is built on device (bf16, rows padded to 256B), then
    h_src / h_dst are fetched feature-major with transpose-mode dma_gather.
  - Edge MLP: z1 = relu(W1s^T h_src + W1d^T h_dst + Wfe^T fe + b1'), folded
    so he0 is never materialized; he1 = z1 @ We2 + be2 (bf16 out).
  - Segment-sum: per 128-slot column, one-hot(dst_local) matmul accumulated
    into SBUF per-block; node MLP per block afterwards.
"""
import sys

sys.path.insert(0, "/opt/trn_rl_repo")

import numpy as np
import ml_dtypes
from contextlib import ExitStack

import concourse.bass as bass
import concourse.tile as tile
from concourse import bacc, mybir
from concourse.bass_utils import run_bass_kernel_spmd

dt = mybir.dt
bf16 = ml_dtypes.bfloat16

N_NODES = 50000
N_EDGES = 800000
FN, FIN, FE, H, L = 16, 16, 9, 64, 64
NCORES = 8
P = 128
NBLK_TOT = 392            # ceil(50000/128) padded to multiple of 8
NBLK = NBLK_TOT // NCORES  # 49 blocks per core
NPAD = NBLK_TOT * P        # 50176 padded nodes
NODES_CORE = NBLK * P      # 6272 nodes per core
HALF = 32768               # int16 gather index limit
NCOL_ST = 32               # columns per supertile

_CACHE = {}


def _wrap16(vals):
    """[S] -> [128, S/16] int16 index layout for dma_gather (8x replicated)."""
    S = len(vals)
    assert S % 16 == 0
    a = np.ascontiguousarray(vals.reshape(S // 16, 16).T.astype(np.int16))
    return np.tile(a, (8, 1))


def _build_nc(K_lo, K_hi):
    """One SPMD program; all structure identical across cores."""
    K = K_lo + K_hi
    COLS = NBLK * K            # columns per core
    S = COLS * P               # edge slots per core
    LO_COLS = NBLK * K_lo      # columns in the lo phase

    def block_of(c):
        if c < LO_COLS:
            return c // K_lo
        return (c - LO_COLS) // K_hi

    nc = bacc.Bacc("TRN2", target_bir_lowering=False, debug=False)

    def inp(name, shape, d):
        return nc.dram_tensor(name, shape, d, kind="ExternalInput").ap()

    xt_g = inp("xt_g", [FN + FIN, NPAD], dt.bfloat16)
    xt_o = inp("xt_o", [FN + FIN, NODES_CORE], dt.bfloat16)
    fet = inp("fet", [FE, S], dt.bfloat16)
    srcw = inp("srcw", [P, S // 16], dt.int16)
    dstw = inp("dstw", [P, S // 16], dt.int16)
    dstloc = inp("dstloc", [P, COLS], dt.bfloat16)
    iota = inp("iota", [P, P], dt.bfloat16)
    w1s = inp("w1s", [H, H], dt.bfloat16)
    w1d = inp("w1d", [H, H], dt.bfloat16)
    wfe = inp("wfe", [FE, H], dt.bfloat16)
    b1p = inp("b1p", [H, 1], dt.float32)
    we2 = inp("we2", [H, L], dt.bfloat16)
    be2b = inp("be2b", [P, L], dt.float32)
    wn = inp("wn", [FN + FIN, H], dt.bfloat16)
    bnb = inp("bnb", [P, H], dt.float32)
    bnT = inp("bnT", [H, 1], dt.float32)
    wu1a = inp("wu1a", [H, H], dt.bfloat16)
    wu1b = inp("wu1b", [L, H], dt.bfloat16)
    bu1 = inp("bu1", [H, 1], dt.float32)
    wu2 = inp("wu2", [H, L], dt.bfloat16)
    bu2b = inp("bu2b", [P, L], dt.float32)

    he1_out = nc.dram_tensor("he1_out", [P, COLS, L], dt.bfloat16, kind="ExternalOutput").ap()
    hn1_out = nc.dram_tensor("hn1_out", [P, NBLK, L], dt.float32, kind="ExternalOutput").ap()

    g_tab = nc.dram_tensor("g_tab", [NPAD, 2 * H], dt.bfloat16).ap()
    d_tab = nc.dram_tensor("d_tab", [NODES_CORE, 2 * H], dt.bfloat16).ap()

    with tile.TileContext(nc) as tc, ExitStack() as ctx:
        const = ctx.enter_context(tc.tile_pool(name="const", bufs=1))
        sb = ctx.enter_context(tc.tile_pool(name="sb", bufs=2))
        sbg = ctx.enter_context(tc.tile_pool(name="sbg", bufs=2))
        sbsmall = ctx.enter_context(tc.tile_pool(name="sbsmall", bufs=3))
        ps_z1 = ctx.enter_context(tc.tile_pool(name="ps_z1", bufs=2, space="PSUM"))
        ps_he1 = ctx.enter_context(tc.tile_pool(name="ps_he1", bufs=2, space="PSUM"))
        ps_agg = ctx.enter_context(tc.tile_pool(name="ps_agg", bufs=2, space="PSUM"))
        ps_nd = ctx.enter_context(tc.tile_pool(name="ps_nd", bufs=2, space="PSUM"))

        # ---- constants in SBUF ----
        def load_const(ap_in, shape, d):
            t = const.tile(shape, d, tag=f"c_{ap_in.tensor.name}")
            nc.sync.dma_start(t[:], ap_in[:])
            return t

        w1s_t = load_const(w1s, [H, H], dt.bfloat16)
        w1d_t = load_const(w1d, [H, H], dt.bfloat16)
        wfe_t = load_const(wfe, [FE, H], dt.bfloat16)
        b1p_t = load_const(b1p, [H, 1], dt.float32)
        we2_t = load_const(we2, [H, L], dt.bfloat16)
        be2b_t = load_const(be2b, [P, L], dt.float32)
        wn_t = load_const(wn, [FN + FIN, H], dt.bfloat16)
        bnb_t = load_const(bnb, [P, H], dt.float32)
        bnT_t = load_const(bnT, [H, 1], dt.float32)
        wu1a_t = load_const(wu1a, [H, H], dt.bfloat16)
        wu1b_t = load_const(wu1b, [L, H], dt.bfloat16)
        bu1_t = load_const(bu1, [H, 1], dt.float32)
        wu2_t = load_const(wu2, [H, L], dt.bfloat16)
        bu2b_t = load_const(bu2b, [P, L], dt.float32)
        iota_t = load_const(iota, [P, P], dt.bfloat16)
        dstloc_t = load_const(dstloc, [P, COLS], dt.bfloat16)
        srcw_t = load_const(srcw, [P, S // 16], dt.int16)
        dstw_t = load_const(dstw, [P, S // 16], dt.int16)

        zero64 = const.tile([P, H], dt.bfloat16, tag="zero64")
        nc.gpsimd.memset(zero64[:], 0)
        ident = const.tile([P, P], dt.bfloat16, tag="ident")
        from concourse.masks import make_identity
        make_identity(nc, ident[:])

        hn0T_own = const.tile([H, NODES_CORE], dt.bfloat16, tag="hn0T_own")
        agg_sb = const.tile([P, NBLK * L], dt.float32, tag="agg_sb")
        nc.gpsimd.memset(agg_sb[:], 0)
        hn1_sb = const.tile([P, NBLK * L], dt.float32, tag="hn1_sb")

        # ---- phase A: node projection tables ----
        ATILE = 16  # node tiles per xt load chunk
        for t0 in range(0, NBLK_TOT, ATILE):
            t1 = min(t0 + ATILE, NBLK_TOT)
            xg = sbsmall.tile([FN + FIN, ATILE * P], dt.bfloat16, tag="xg")
            nc.sync.dma_start(xg[:, : (t1 - t0) * P], xt_g[:, t0 * P : t1 * P])
            for t in range(t0, t1):
                hp = ps_nd.tile([P, H], dt.float32, space="PSUM", tag="nd")
                nc.tensor.matmul(
                    out=hp[:], lhsT=xg[:, (t - t0) * P : (t - t0 + 1) * P],
                    rhs=wn_t[:], start=True, stop=True)
                hb = sbsmall.tile([P, H], dt.bfloat16, tag="hb")
                nc.vector.tensor_add(out=hb[:], in0=hp[:], in1=bnb_t[:])
                nc.sync.dma_start(g_tab[t * P : (t + 1) * P, 0:H], hb[:])
                nc.sync.dma_start(g_tab[t * P : (t + 1) * P, H : 2 * H], zero64[:])

        for t in range(NBLK):
            xo = sbsmall.tile([FN + FIN, P], dt.bfloat16, tag="xo")
            nc.sync.dma_start(xo[:], xt_o[:, t * P : (t + 1) * P])
            hp = ps_nd.tile([P, H], dt.float32, space="PSUM", tag="nd")
            nc.tensor.matmul(out=hp[:], lhsT=xo[:], rhs=wn_t[:], start=True, stop=True)
            hb = sbsmall.tile([P, H], dt.bfloat16, tag="hb")
            nc.vector.tensor_add(out=hb[:], in0=hp[:], in1=bnb_t[:])
            nc.sync.dma_start(d_tab[t * P : (t + 1) * P, 0:H], hb[:])
            nc.sync.dma_start(d_tab[t * P : (t + 1) * P, H : 2 * H], zero64[:])
            # own-range hn0T (feature-major) for the node MLP
            hTp = ps_nd.tile([H, P], dt.float32, space="PSUM", tag="nd")
            nc.tensor.matmul(out=hTp[:], lhsT=wn_t[:], rhs=xo[:], start=True, stop=True)
            nc.scalar.activation(
                hn0T_own[:, t * P : (t + 1) * P], hTp[:],
                mybir.ActivationFunctionType.Identity, bias=bnT_t[:])

        # ---- phase B: edge supertiles ----
        n_st = (COLS + NCOL_ST - 1) // NCOL_ST
        for st in range(n_st):
            c0 = st * NCOL_ST
            c1 = min(c0 + NCOL_ST, COLS)
            ncol = c1 - c0
            ns = ncol * P

            XTs = sbg.tile([P, 1, NCOL_ST * P], dt.bfloat16, tag="XTs")
            XTd = sbg.tile([P, 1, NCOL_ST * P], dt.bfloat16, tag="XTd")
            # src gather: split at the lo/hi phase boundary column
            ranges = []
            if c1 <= LO_COLS or c0 >= LO_COLS:
                ranges.append((c0, c1, 0 if c1 <= LO_COLS else 1))
            else:
                ranges.append((c0, LO_COLS, 0))
                ranges.append((LO_COLS, c1, 1))
            for (a, b, hi) in ranges:
                nsl = (b - a) * P
                src_slice = srcw_t[:, a * 8 : b * 8]
                tab = g_tab[HALF:, :] if hi else g_tab[0:HALF, :]
                nc.gpsimd.dma_gather(
                    XTs[:, :, (a - c0) * P : (b - c0) * P], tab, src_slice,
                    nsl, nsl, 2 * H, transpose=True, single_packet=False)
            nc.gpsimd.dma_gather(
                XTd[:, :, 0:ns], d_tab[:], dstw_t[:, c0 * 8 : c1 * 8],
                ns, ns, 2 * H, transpose=True, single_packet=False)

            fe_t = sb.tile([FE, NCOL_ST * P], dt.bfloat16, tag="fe")
            nc.sync.dma_start(fe_t[:, 0:ns], fet[:, c0 * P : c1 * P])

            he1_sb = sb.tile([P, NCOL_ST * L], dt.bfloat16, tag="he1")

            for t in range(0, ncol, 2):  # 256-slot tiles (2 columns)
                tn = min(2, ncol - t)
                w = tn * P
                sl = slice(t * P, t * P + w)
                z1 = ps_z1.tile([H, 2 * P], dt.float32, space="PSUM", tag="z1")
                nc.tensor.matmul(out=z1[:, :w], lhsT=w1s_t[:], rhs=XTs[0:H, 0, sl],
                                 start=True, stop=False)
                nc.tensor.matmul(out=z1[:, :w], lhsT=w1d_t[:], rhs=XTd[0:H, 0, sl],
                                 start=False, stop=False)
                nc.tensor.matmul(out=z1[:, :w], lhsT=wfe_t[:], rhs=fe_t[:, sl],
                                 start=False, stop=True)
                z1s = sbsmall.tile([H, 2 * P], dt.bfloat16, tag="z1s")
                nc.scalar.activation(z1s[:, :w], z1[:, :w],
                                     mybir.ActivationFunctionType.Relu, bias=b1p_t[:])
                for h in range(tn):
                    c = c0 + t + h
                    hp = ps_he1.tile([P, L], dt.float32, space="PSUM", tag="hep")
                    nc.tensor.matmul(out=hp[:], lhsT=z1s[:, h * P : (h + 1) * P],
                                     rhs=we2_t[:], start=True, stop=True)
                    he_sl = he1_sb[:, (t + h) * L : (t + h + 1) * L]
                    nc.vector.tensor_add(out=he_sl, in0=hp[:], in1=be2b_t[:])
                    oh = sbsmall.tile([P, P], dt.bfloat16, tag="oh")
                    nc.vector.tensor_tensor(
                        out=oh[:], in0=dstloc_t[:, c : c + 1].to_broadcast([P, P]),
                        in1=iota_t[:], op=mybir.AluOpType.is_equal)
                    ap_ = ps_agg.tile([P, L], dt.float32, space="PSUM", tag="aggp")
                    nc.tensor.matmul(out=ap_[:], lhsT=oh[:], rhs=he_sl,
                                     start=True, stop=True)
                    b = block_of(c)
                    nc.vector.tensor_add(
                        out=agg_sb[:, b * L : (b + 1) * L],
                        in0=agg_sb[:, b * L : (b + 1) * L], in1=ap_[:])

            nc.sync.dma_start(
                he1_out[:, c0:c1, :],
                he1_sb[:, 0 : ncol * L].rearrange("p (c l) -> p c l", l=L))

        # ---- phase C: node MLP per block ----
        for b in range(NBLK):
            agb = sbsmall.tile([P, L], dt.bfloat16, tag="agb")
            nc.vector.tensor_copy(agb[:], agg_sb[:, b * L : (b + 1) * L])
            aTp = ps_nd.tile([L, P], dt.bfloat16, space="PSUM", tag="nd")
            nc.tensor.transpose(out=aTp[:], in_=agb[:], identity=ident[:])
            aT = sbsmall.tile([L, P], dt.bfloat16, tag="aT")
            nc.vector.tensor_copy(aT[:], aTp[:])
            z1u = ps_nd.tile([H, P], dt.float32, space="PSUM", tag="nd")
            nc.tensor.matmul(out=z1u[:], lhsT=wu1a_t[:],
                             rhs=hn0T_own[:, b * P : (b + 1) * P], start=True, stop=False)
            nc.tensor.matmul(out=z1u[:], lhsT=wu1b_t[:], rhs=aT[:], start=False, stop=True)
            z1us = sbsmall.tile([H, P], dt.bfloat16, tag="z1us")
            nc.scalar.activation(z1us[:], z1u[:],
                                 mybir.ActivationFunctionType.Relu, bias=bu1_t[:])
            h1p = ps_nd.tile([P, L], dt.float32, space="PSUM", tag="nd")
            nc.tensor.matmul(out=h1p[:], lhsT=z1us[:], rhs=wu2_t[:], start=True, stop=True)
            nc.vector.tensor_add(out=hn1_sb[:, b * L : (b + 1) * L],
                                 in0=h1p[:], in1=bu2b_t[:])
        nc.sync.dma_start(
            hn1_out[:],
            hn1_sb[:].rearrange("p (b l) -> p b l", l=L))

    nc.compile()
    return nc


def _prep(fn, hn, fe, edge_index, Wn, bn, We, be, We1, be1, We2, be2,
          Wu1, bu1, Wu2, bu2):
    src = np.asarray(edge_index[0], dtype=np.int64)
    dst = np.asarray(edge_index[1], dtype=np.int64)
    E = src.shape[0]

    blk = (dst // P).astype(np.int64)          # 0..390
    hi = (src >= HALF).astype(np.int64)        # lo/hi phase by src
    # rank within (block, half)
    key = blk * 2 + hi
    order = np.argsort(key, kind="stable")
    ks = key[order]
    grp_start = np.r_[0, np.flatnonzero(np.diff(ks)) + 1]
    start_per_edge = np.zeros(E, np.int64)
    start_per_edge[grp_start] = grp_start
    start_per_edge = np.maximum.accumulate(start_per_edge)
    rank_sorted = np.arange(E) - start_per_edge
    rank = np.empty(E, np.int64)
    rank[order] = rank_sorted

    cnt = np.bincount(key, minlength=NBLK_TOT * 2)
    cnt_lo = cnt[0::2][:NBLK_TOT]
    cnt_hi = cnt[1::2][:NBLK_TOT]
    K_lo = max(1, int(np.ceil(cnt_lo.max() / P)))
    K_hi = max(1, int(np.ceil(cnt_hi.max() / P)))
    if (K_lo + K_hi) % 2:
        K_hi += 1
    K = K_lo + K_hi
    COLS = NBLK * K
    S = COLS * P
    LO_COLS = NBLK * K_lo

    # slot of each edge within its core
    core = blk // NBLK
    bl = blk % NBLK
    slot = np.where(hi == 0,
                    bl * K_lo * P + rank,
                    LO_COLS * P + bl * K_hi * P + rank)

    col_block = np.empty(COLS, np.int64)
    cidx = np.arange(COLS)
    col_block[cidx < LO_COLS] = cidx[cidx < LO_COLS] // K_lo
    col_block[cidx >= LO_COLS] = (cidx[cidx >= LO_COLS] - LO_COLS) // K_hi

    x = np.concatenate([np.asarray(fn), np.asarray(hn)], axis=1).astype(np.float32)
    xpad = np.zeros((NPAD, FN + FIN), np.float32)
    xpad[:N_NODES] = x
    xt_g = np.ascontiguousarray(xpad.T).astype(bf16)

    We1 = np.asarray(We1, np.float32)
    We = np.asarray(We, np.float32)
    wfe_f = We @ We1[2 * H :]
    b1p = (np.asarray(be1, np.float32) + np.asarray(be, np.float32) @ We1[2 * H :])

    fe_np = np.asarray(fe, np.float32)

    common = dict(
        iota=np.tile(np.arange(P, dtype=np.float32)[None, :], (P, 1)).astype(bf16),
        w1s=np.ascontiguousarray(We1[0:H]).astype(bf16),
        w1d=np.ascontiguousarray(We1[H : 2 * H]).astype(bf16),
        wfe=np.ascontiguousarray(wfe_f).astype(bf16),
        b1p=b1p.reshape(H, 1).astype(np.float32),
        we2=np.asarray(We2, np.float32).astype(bf16),
        be2b=np.tile(np.asarray(be2, np.float32)[None, :], (P, 1)),
        wn=np.asarray(Wn, np.float32).astype(bf16),
        bnb=np.tile(np.asarray(bn, np.float32)[None, :], (P, 1)),
        bnT=np.asarray(bn, np.float32).reshape(H, 1),
        wu1a=np.ascontiguousarray(np.asarray(Wu1, np.float32)[0:H]).astype(bf16),
        wu1b=np.ascontiguousarray(np.asarray(Wu1, np.float32)[H : H + L]).astype(bf16),
        bu1=np.asarray(bu1, np.float32).reshape(H, 1),
        wu2=np.asarray(Wu2, np.float32).astype(bf16),
        bu2b=np.tile(np.asarray(bu2, np.float32)[None, :], (P, 1)),
        xt_g=xt_g,
    )

    in_maps = []
    slot_edge_all = []
    for c in range(NCORES):
        m = core == c
        e_ids = np.flatnonzero(m)
        sl = slot[m]
        slot_edge = np.full(S, -1, np.int64)
        slot_edge[sl] = e_ids
        slot_edge_all.append(slot_edge)

        src_v = np.zeros(S, np.int64)
        src_v[sl] = np.where(hi[m] == 1, src[m] - HALF, src[m])
        dst_v = np.zeros(S, np.int64)
        node0 = c * NODES_CORE
        dst_v[sl] = dst[m] - node0

        # dst_local = dst - (node0 + block*128)
        dl = np.full(S, 300.0, np.float32)
        dl[sl] = dst[m] - node0 - col_block[sl // P] * P
        dstloc = np.ascontiguousarray(
            dl.reshape(COLS, P).T).astype(bf16)

        fet = np.zeros((S, FE), np.float32)
        fet[sl] = fe_np[m]
        fet = np.ascontiguousarray(fet.T).astype(bf16)

        xo = xpad[node0 : node0 + NODES_CORE]
        xt_o = np.ascontiguousarray(xo.T).astype(bf16)

        in_maps.append(dict(
            common,
            xt_o=xt_o,
            fet=fet,
            srcw=_wrap16(src_v),
            dstw=_wrap16(dst_v),
            dstloc=dstloc,
        ))

    return K_lo, K_hi, in_maps, slot_edge_all


def kernel(**inputs):
    K_lo, K_hi, in_maps, slot_edge_all = _prep(**inputs)
    key = (K_lo, K_hi)
    if key not in _CACHE:
        _CACHE[key] = _build_nc(K_lo, K_hi)
    nc = _CACHE[key]
    res = run_bass_kernel_spmd(nc, in_maps, list(range(NCORES)))

    K = K_lo + K_hi
    COLS = NBLK * K
    S = COLS * P
    he1_full = np.zeros((N_EDGES, L), np.float32)
    hn1_parts = []
    for c in range(NCORES):
        r = res.results[c]
        he1_c = r["he1_out"].astype(np.float32).transpose(1, 0, 2).reshape(S, L)
        se = slot_edge_all[c]
        valid = se >= 0
        he1_full[se[valid]] = he1_c[valid]
        hn1_parts.append(
            r["hn1_out"].astype(np.float32).transpose(1, 0, 2).reshape(NODES_CORE, L))
    hn1_full = np.concatenate(hn1_parts, axis=0)[:N_NODES]
    return hn1_full, he1_full
